# revision 1
# baseline (speedup 1.0000x reference)
"""Cost-volume kernel for Trainium2 (8 NeuronCores, data-parallel over B*H rows).

cost[b,h,w,d] = mean_c left[b,h,w,c] * right[b,h,w-(d+1),c], 0 where w-d-1 < 0
Shapes: B=4, H=256, W=512, C=64, D=64 (f32).

Strategy per core (128 independent (b,h) rows; kernel() uses build_nc_m64):
  - Host pre-truncates inputs to bf16-in-f32 (low mantissa zeroed; left also
    pre-scaled by 1/C) so an f32->bf16 *bitcast view* has real bf16 values in
    odd 16-bit lanes and exact +0.0 in even lanes.
  - dma_start_transpose loads [RG*512, 128]bf16 -> SBUF [128, RG*512]: channel
    bytes land on partitions (odd partitions = data, even = 0).
  - TensorE banded matmuls, K=128 (zero lanes drop out, so the contraction is
    sum_c (L/64)*R = mean_c L*R). Each 128-wide w block is two M=64 matmuls
    packed into psum partition halves via tile_position col groups, giving a
    [128, 127] psum rect per block whose band sits at col (p mod 64) + d'.
  - DVE/ACT evict psum (f32 -> bf16) into a per-group SBUF rect; the rect is
    stored contiguously to per-group DRAM scratch, and a DRAM->DRAM DMA with
    a sheared (flat-affine) source AP pulls out the band
    G[w, d'] = rect[p, 127*m + (p mod 64) + d']   (d' = 63 - d).
    (SBUF-side DMA descriptors cannot express the +1-element-per-partition
    shear - DRAM-side flat addressing can.)
  - Host flips d' -> d and casts bf16 -> f32 during unsharding.
HW time ~150-170 us/core vs ~140 us memory roofline (50 MB/core at 358 GB/s).
"""

import numpy as np

N_CORES = 8
B_FULL, H_FULL, W, C = 4, 256, 512, 64
D = 64
ROWS = B_FULL * H_FULL          # 1024 independent rows
ROWS_PER_CORE = ROWS // N_CORES  # 128
CB = 2 * C                       # bf16 lanes per w after bitcast
RG = 8                           # rows per group (1MB transpose DMAs)
NBLK = W // 128                  # w-blocks per row
N_WIN = 191                      # matmul rhs window
RECTB = 191                      # bf16 cols per block in the rect buffer


def build_nc(rows=ROWS_PER_CORE, pair_evicts=True, store_eng="scalar",
             diag_eng="scalar", lt_bufs=3, rect_bufs=3, ps_bufs=6, repeat=1,
             m64=False):
    if m64:
        return build_nc_m64(rows=rows, store_eng=store_eng, diag_eng=diag_eng,
                            lt_bufs=lt_bufs, rect_bufs=rect_bufs,
                            ps_bufs=ps_bufs, repeat=repeat)
    import concourse.bass as bass
    import concourse.mybir as mybir
    import concourse.tile as tile
    from concourse import bacc

    nc = bacc.Bacc()
    left = nc.declare_dram_parameter("left", [rows * W, C], mybir.dt.float32,
                                     isOutput=False)
    right = nc.declare_dram_parameter("right", [rows * W, C], mybir.dt.float32,
                                      isOutput=False)
    out = nc.declare_dram_parameter("out", [rows * W, D], mybir.dt.bfloat16,
                                    isOutput=True)

    ng = rows // RG
    nblocks = RG * NBLK            # rect blocks per group
    bcols = nblocks * RECTB        # rect buffer bf16 cols

    # per-group DRAM scratch for the full rects (band + waste); the diagonal
    # band is then pulled out with a DRAM->DRAM DMA (flat addressing allows
    # the +1-element-per-row shear that SBUF-side descriptors cannot express).
    scr = [nc.dram_tensor(f"scr{g}", [128, bcols], mybir.dt.bfloat16)
           for g in range(ng)]

    with tile.TileContext(nc) as tc:
        with (
            tc.tile_pool(name="lt", bufs=lt_bufs) as lt_pool,
            tc.tile_pool(name="rt", bufs=lt_bufs) as rt_pool,
            tc.tile_pool(name="rect", bufs=rect_bufs) as rect_pool,
            tc.tile_pool(name="ps", bufs=ps_bufs, space="PSUM") as psum_pool,
        ):
          for _rep in range(repeat):
            for g in range(ng):
                row0 = g * RG
                Lt = lt_pool.tile([CB, RG * W], mybir.dt.bfloat16, tag="lt")
                Rt = rt_pool.tile([CB, RG * W], mybir.dt.bfloat16, tag="rt")
                lsrc = left[row0 * W:(row0 + RG) * W, :].bitcast(mybir.dt.bfloat16)
                rsrc = right[row0 * W:(row0 + RG) * W, :].bitcast(mybir.dt.bfloat16)
                nc.sync.dma_start(Lt[:, :], lsrc, transpose=True)
                nc.sync.dma_start(Rt[:, :], rsrc, transpose=True)

                Brect = rect_pool.tile([128, bcols], mybir.dt.bfloat16, tag="rect")
                for r in range(RG):
                    # two psum blocks share one bank slot: [128, 382]
                    for half in range(NBLK // 2):
                        i0, i1 = 2 * half, 2 * half + 1
                        m0 = r * NBLK + i0
                        col0 = m0 * RECTB
                        P = psum_pool.tile([128, 2 * N_WIN], mybir.dt.float32,
                                           tag="ps")
                        lhsT0 = Lt[:, r * W + i0 * 128: r * W + (i0 + 1) * 128]
                        lhsT1 = Lt[:, r * W + i1 * 128: r * W + (i1 + 1) * 128]
                        w1 = r * W + i1 * 128
                        rhs1 = Rt[:, w1 - 64: w1 + 127]
                        if i0 == 0:
                            rhs0 = Rt[:, r * W: r * W + 127]
                            nc.tensor.matmul(P[:, 64:191], lhsT0, rhs0,
                                             start=True, stop=True)
                            nc.tensor.matmul(P[:, 191:382], lhsT1, rhs1,
                                             start=True, stop=True)
                            nc.gpsimd.memset(Brect[:, col0: col0 + 64], 0.0)
                            ev_src = P[:, 64:382]
                            ev_dst = Brect[:, col0 + 64: col0 + 2 * RECTB]
                        else:
                            w0 = r * W + i0 * 128
                            rhs0 = Rt[:, w0 - 64: w0 + 127]
                            nc.tensor.matmul(P[:, 0:191], lhsT0, rhs0,
                                             start=True, stop=True)
                            nc.tensor.matmul(P[:, 191:382], lhsT1, rhs1,
                                             start=True, stop=True)
                            ev_src = P[:, :]
                            ev_dst = Brect[:, col0: col0 + 2 * RECTB]
                        if half % 2 == 0:
                            nc.vector.tensor_copy(ev_dst, ev_src)
                        else:
                            nc.scalar.copy(ev_dst, ev_src)

                # rect -> DRAM scratch (contiguous), then band extraction
                # G[128m + p, d'] = scr[p, RECTB*m + p + d'] via DRAM->DRAM.
                bap = Brect[:, :]
                sap = scr[g][:, :]
                oap = out[row0 * W:(row0 + RG) * W, :]
                getattr(nc, store_eng).dma_start(scr[g][:, :], Brect[:, :])
                src = bass.AP(sap.tensor, sap.offset,
                              [[bcols + 1, 128], [RECTB, nblocks], [1, D]])
                dst = bass.AP(oap.tensor, oap.offset,
                              [[D, 128], [128 * D, nblocks], [1, D]])
                getattr(nc, diag_eng).dma_start(dst, src)

    nc.compile()
    return nc


RECT2 = 127   # rect cols per block in the m64 layout


def build_nc_m64(rows=ROWS_PER_CORE, store_eng="sync", diag_eng="scalar",
                 lt_bufs=3, rect_bufs=3, ps_bufs=6, repeat=1, msub=64, rg=None,
                 store_split=1, qsplit=False, rt_eng="sync"):
    """M=64 col-group variant: each 128-w block is two M=64 matmuls packed
    into psum partition halves via tile_position, so the rect narrows to
    127 cols (band at col (p mod 64) + d) and scratch traffic drops 33%."""
    import concourse.bass as bass
    import concourse.mybir as mybir
    import concourse.tile as tile
    from concourse import bacc

    nc = bacc.Bacc()
    left = nc.declare_dram_parameter("left", [rows * W, C], mybir.dt.float32,
                                     isOutput=False)
    right = nc.declare_dram_parameter("right", [rows * W, C], mybir.dt.float32,
                                      isOutput=False)
    out = nc.declare_dram_parameter("out", [rows * W, D], mybir.dt.bfloat16,
                                    isOutput=True)

    RG = rg or globals()["RG"]
    ng = rows // RG
    nblocks = RG * NBLK
    rect2 = msub + 63              # rect cols per block
    nsub = 128 // msub             # sub-matmuls per block
    bcols = nblocks * rect2
    SW = 95                        # stored cols per block under qsplit
    scols = nblocks * SW
    scr = [nc.dram_tensor(f"scr{g}", [128, scols if qsplit else bcols],
                          mybir.dt.bfloat16)
           for g in range(ng)]

    with tile.TileContext(nc) as tc:
        with (
            tc.tile_pool(name="lt", bufs=lt_bufs) as lt_pool,
            tc.tile_pool(name="rt", bufs=lt_bufs) as rt_pool,
            tc.tile_pool(name="rect", bufs=rect_bufs) as rect_pool,
            tc.tile_pool(name="ps", bufs=ps_bufs, space="PSUM") as psum_pool,
        ):
          for _rep in range(repeat):
            for g in range(ng):
                row0 = g * RG
                Lt = lt_pool.tile([CB, RG * W], mybir.dt.bfloat16, tag="lt")
                Rt = rt_pool.tile([CB, RG * W], mybir.dt.bfloat16, tag="rt")
                lsrc = left[row0 * W:(row0 + RG) * W, :].bitcast(mybir.dt.bfloat16)
                rsrc = right[row0 * W:(row0 + RG) * W, :].bitcast(mybir.dt.bfloat16)
                nc.sync.dma_start(Lt[:, :], lsrc, transpose=True)
                getattr(nc, rt_eng).dma_start(Rt[:, :], rsrc, transpose=True)

                Brect = rect_pool.tile([128, bcols], mybir.dt.bfloat16, tag="rect")
                for r in range(RG):
                    for half in range(NBLK // 2):
                        i0, i1 = 2 * half, 2 * half + 1
                        m0 = r * NBLK + i0
                        col0 = m0 * rect2
                        P = psum_pool.tile([128, 512], mybir.dt.float32,
                                           tag="ps")
                        for bi, i in enumerate((i0, i1)):
                            wl = i * 128            # row-local block base
                            pc = bi * rect2
                            rb = r * W
                            # sub-block s: w in [wl+s*msub, wl+(s+1)*msub),
                            # psum partitions [s*msub, (s+1)*msub), window
                            # w' in [wl+s*msub-64, wl+s*msub+rect2-65].
                            for s in range(nsub):
                                ws = wl + s * msub
                                p0, p1 = s * msub, (s + 1) * msub
                                if i == 0 and ws < 64:
                                    # w' < 0 head undefined: zero it, compute
                                    # the valid tail starting at w' = 0
                                    zc = 64 - ws
                                    nc.vector.memset(P[p0:p1, pc: pc + zc], 0.0)
                                    nc.tensor.matmul(
                                        P[p0:p1, pc + zc: pc + rect2],
                                        Lt[:, rb + ws: rb + ws + msub],
                                        Rt[:, rb: rb + rect2 - zc],
                                        start=True, stop=True,
                                        tile_position=(0, p0))
                                else:
                                    nc.tensor.matmul(
                                        P[p0:p1, pc: pc + rect2],
                                        Lt[:, rb + ws: rb + ws + msub],
                                        Rt[:, rb + ws - 64: rb + ws + rect2 - 64],
                                        start=True, stop=True,
                                        tile_position=(0, p0))
                        ev_dst = Brect[:, col0: col0 + 2 * rect2]
                        if half % 2 == 0:
                            nc.vector.tensor_copy(ev_dst, P[:, 0:2 * rect2])
                        else:
                            nc.scalar.copy(ev_dst, P[:, 0:2 * rect2])


                sap = scr[g][:, :]
                bap2 = Brect[:, :]
                oap = out[row0 * W:(row0 + RG) * W, :]
                if qsplit:
                    # store only the 95-col window each partition quarter's
                    # band needs: quarter t = partitions [32t, 32t+32),
                    # q = p mod 64 = 32*(t%2) + u, window cols
                    # [32*(t%2), 32*(t%2)+95) of each block ->
                    # scr[32t+u, SW*m + u + d] holds the band at col u+d.
                    for t in range(4):
                        ct = 32 * (t % 2)
                        bs = bass.AP(bap2.tensor,
                                     bap2.offset + 32 * t * bcols + ct,
                                     [[bcols, 32], [rect2, nblocks], [1, SW]])
                        sd = bass.AP(sap.tensor,
                                     sap.offset + 32 * t * scols,
                                     [[scols, 32], [SW, nblocks], [1, SW]])
                        getattr(nc, store_eng).dma_start(sd, bs)
                    for t in range(4):
                        srcp = bass.AP(sap.tensor,
                                       sap.offset + 32 * t * scols,
                                       [[scols + 1, 32], [SW, nblocks], [1, D]])
                        dstp = bass.AP(oap.tensor,
                                       oap.offset + 32 * t * D,
                                       [[D, 32], [128 * D, nblocks], [1, D]])
                        getattr(nc, diag_eng).dma_start(dstp, srcp)
                else:
                    # store rect (in store_split chunks of rows), then extract
                    # per (chunk, partition half): G[128m + msub*h + q, d'] =
                    # scr[msub*h + q, rect2*m + q + d']
                    rpc_ = RG // store_split          # rows per chunk
                    bpc = rpc_ * NBLK                 # blocks per chunk
                    ccols = bpc * rect2               # rect cols per chunk
                    for cch in range(store_split):
                        getattr(nc, store_eng).dma_start(
                            scr[g][:, cch * ccols:(cch + 1) * ccols],
                            Brect[:, cch * ccols:(cch + 1) * ccols])
                        for h in range(nsub):
                            src = bass.AP(sap.tensor,
                                          sap.offset + h * msub * bcols + cch * ccols,
                                          [[bcols + 1, msub], [rect2, bpc], [1, D]])
                            dst = bass.AP(oap.tensor,
                                          oap.offset + h * msub * D
                                          + cch * bpc * 128 * D,
                                          [[D, msub], [128 * D, bpc], [1, D]])
                            getattr(nc, diag_eng).dma_start(dst, src)

    nc.compile()
    return nc


def _prep(x, scale):
    # exact scale (power of two), then truncate to bf16-in-f32: the low 16
    # bits of every f32 become zero, so the bf16 *pair* view has +0.0 in the
    # garbage lanes.
    x = np.ascontiguousarray(x, dtype=np.float32) * scale
    return (x.view(np.uint32) & np.uint32(0xFFFF0000)).view(np.float32)


_NC_CACHE = {}


def kernel(left_feature, right_feature):
    from concourse.bass_utils import run_bass_kernel_spmd

    lf = _prep(left_feature, 1.0 / C).reshape(ROWS, W, C)
    rf = _prep(right_feature, 1.0).reshape(ROWS, W, C)

    if "nc" not in _NC_CACHE:
        _NC_CACHE["nc"] = build_nc_m64(store_eng="sync", diag_eng="sync")
    nc = _NC_CACHE["nc"]

    in_maps = []
    for k in range(N_CORES):
        sl = slice(k * ROWS_PER_CORE, (k + 1) * ROWS_PER_CORE)
        in_maps.append({
            "left": np.ascontiguousarray(lf[sl]).reshape(ROWS_PER_CORE * W, C),
            "right": np.ascontiguousarray(rf[sl]).reshape(ROWS_PER_CORE * W, C),
        })

    res = run_bass_kernel_spmd(nc, in_maps, core_ids=list(range(N_CORES)))

    out = np.empty((ROWS, W, D), dtype=np.float32)
    for k in range(N_CORES):
        g = res.results[k]["out"].astype(np.float32).reshape(ROWS_PER_CORE, W, D)
        out[k * ROWS_PER_CORE:(k + 1) * ROWS_PER_CORE] = g[:, :, ::-1]
    return out.reshape(B_FULL, H_FULL, W, D)



# revision 2
# speedup vs baseline: 642.5292x; 642.5292x over previous
"""Cost-volume kernel for Trainium2 (8 NeuronCores, data-parallel over B*H rows).

cost[b,h,w,d] = mean_c left[b,h,w,c] * right[b,h,w-(d+1),c], 0 where w-d-1 < 0
Shapes: B=4, H=256, W=512, C=64, D=64 (f32).

Strategy per core (128 independent (b,h) rows; kernel() uses build_nc_m64):
  - Host pre-truncates inputs to bf16-in-f32 (low mantissa zeroed; left also
    pre-scaled by 1/C) so an f32->bf16 *bitcast view* has real bf16 values in
    odd 16-bit lanes and exact +0.0 in even lanes.
  - dma_start_transpose loads [RG*512, 128]bf16 -> SBUF [128, RG*512]: channel
    bytes land on partitions (odd partitions = data, even = 0).
  - TensorE banded matmuls, K=128 (zero lanes drop out, so the contraction is
    sum_c (L/64)*R = mean_c L*R). Each 128-wide w block is two M=64 matmuls
    packed into psum partition halves via tile_position col groups, giving a
    [128, 127] psum rect per block whose band sits at col (p mod 64) + d'.
  - DVE/ACT evict psum (f32 -> bf16) into a per-group SBUF rect; the rect is
    stored contiguously to per-group DRAM scratch, and a DRAM->DRAM DMA with
    a sheared (flat-affine) source AP pulls out the band
    G[w, d'] = rect[p, 127*m + (p mod 64) + d']   (d' = 63 - d).
    (SBUF-side DMA descriptors cannot express the +1-element-per-partition
    shear - DRAM-side flat addressing can.)
  - Host flips d' -> d and casts bf16 -> f32 during unsharding.
HW time ~150-170 us/core vs ~140 us memory roofline (50 MB/core at 358 GB/s).
"""

import numpy as np

N_CORES = 8
B_FULL, H_FULL, W, C = 4, 256, 512, 64
D = 64
ROWS = B_FULL * H_FULL          # 1024 independent rows
ROWS_PER_CORE = ROWS // N_CORES  # 128
CB = 2 * C                       # bf16 lanes per w after bitcast
RG = 8                           # rows per group (1MB transpose DMAs)
NBLK = W // 128                  # w-blocks per row
N_WIN = 191                      # matmul rhs window
RECTB = 191                      # bf16 cols per block in the rect buffer


def build_nc(rows=ROWS_PER_CORE, pair_evicts=True, store_eng="scalar",
             diag_eng="scalar", lt_bufs=3, rect_bufs=3, ps_bufs=6, repeat=1,
             m64=False):
    if m64:
        return build_nc_m64(rows=rows, store_eng=store_eng, diag_eng=diag_eng,
                            lt_bufs=lt_bufs, rect_bufs=rect_bufs,
                            ps_bufs=ps_bufs, repeat=repeat)
    import concourse.bass as bass
    import concourse.mybir as mybir
    import concourse.tile as tile
    from concourse import bacc

    nc = bacc.Bacc()
    left = nc.declare_dram_parameter("left", [rows * W, C], mybir.dt.float32,
                                     isOutput=False)
    right = nc.declare_dram_parameter("right", [rows * W, C], mybir.dt.float32,
                                      isOutput=False)
    out = nc.declare_dram_parameter("out", [rows * W, D], mybir.dt.bfloat16,
                                    isOutput=True)

    ng = rows // RG
    nblocks = RG * NBLK            # rect blocks per group
    bcols = nblocks * RECTB        # rect buffer bf16 cols

    # per-group DRAM scratch for the full rects (band + waste); the diagonal
    # band is then pulled out with a DRAM->DRAM DMA (flat addressing allows
    # the +1-element-per-row shear that SBUF-side descriptors cannot express).
    scr = [nc.dram_tensor(f"scr{g}", [128, bcols], mybir.dt.bfloat16)
           for g in range(ng)]

    with tile.TileContext(nc) as tc:
        with (
            tc.tile_pool(name="lt", bufs=lt_bufs) as lt_pool,
            tc.tile_pool(name="rt", bufs=lt_bufs) as rt_pool,
            tc.tile_pool(name="rect", bufs=rect_bufs) as rect_pool,
            tc.tile_pool(name="ps", bufs=ps_bufs, space="PSUM") as psum_pool,
        ):
          for _rep in range(repeat):
            for g in range(ng):
                row0 = g * RG
                Lt = lt_pool.tile([CB, RG * W], mybir.dt.bfloat16, tag="lt")
                Rt = rt_pool.tile([CB, RG * W], mybir.dt.bfloat16, tag="rt")
                lsrc = left[row0 * W:(row0 + RG) * W, :].bitcast(mybir.dt.bfloat16)
                rsrc = right[row0 * W:(row0 + RG) * W, :].bitcast(mybir.dt.bfloat16)
                nc.sync.dma_start(Lt[:, :], lsrc, transpose=True)
                nc.sync.dma_start(Rt[:, :], rsrc, transpose=True)

                Brect = rect_pool.tile([128, bcols], mybir.dt.bfloat16, tag="rect")
                for r in range(RG):
                    # two psum blocks share one bank slot: [128, 382]
                    for half in range(NBLK // 2):
                        i0, i1 = 2 * half, 2 * half + 1
                        m0 = r * NBLK + i0
                        col0 = m0 * RECTB
                        P = psum_pool.tile([128, 2 * N_WIN], mybir.dt.float32,
                                           tag="ps")
                        lhsT0 = Lt[:, r * W + i0 * 128: r * W + (i0 + 1) * 128]
                        lhsT1 = Lt[:, r * W + i1 * 128: r * W + (i1 + 1) * 128]
                        w1 = r * W + i1 * 128
                        rhs1 = Rt[:, w1 - 64: w1 + 127]
                        if i0 == 0:
                            rhs0 = Rt[:, r * W: r * W + 127]
                            nc.tensor.matmul(P[:, 64:191], lhsT0, rhs0,
                                             start=True, stop=True)
                            nc.tensor.matmul(P[:, 191:382], lhsT1, rhs1,
                                             start=True, stop=True)
                            nc.gpsimd.memset(Brect[:, col0: col0 + 64], 0.0)
                            ev_src = P[:, 64:382]
                            ev_dst = Brect[:, col0 + 64: col0 + 2 * RECTB]
                        else:
                            w0 = r * W + i0 * 128
                            rhs0 = Rt[:, w0 - 64: w0 + 127]
                            nc.tensor.matmul(P[:, 0:191], lhsT0, rhs0,
                                             start=True, stop=True)
                            nc.tensor.matmul(P[:, 191:382], lhsT1, rhs1,
                                             start=True, stop=True)
                            ev_src = P[:, :]
                            ev_dst = Brect[:, col0: col0 + 2 * RECTB]
                        if half % 2 == 0:
                            nc.vector.tensor_copy(ev_dst, ev_src)
                        else:
                            nc.scalar.copy(ev_dst, ev_src)

                # rect -> DRAM scratch (contiguous), then band extraction
                # G[128m + p, d'] = scr[p, RECTB*m + p + d'] via DRAM->DRAM.
                bap = Brect[:, :]
                sap = scr[g][:, :]
                oap = out[row0 * W:(row0 + RG) * W, :]
                getattr(nc, store_eng).dma_start(scr[g][:, :], Brect[:, :])
                src = bass.AP(sap.tensor, sap.offset,
                              [[bcols + 1, 128], [RECTB, nblocks], [1, D]])
                dst = bass.AP(oap.tensor, oap.offset,
                              [[D, 128], [128 * D, nblocks], [1, D]])
                getattr(nc, diag_eng).dma_start(dst, src)

    nc.compile()
    return nc


RECT2 = 127   # rect cols per block in the m64 layout


def build_nc_m64(rows=ROWS_PER_CORE, store_eng="sync", diag_eng="scalar",
                 lt_bufs=3, rect_bufs=3, ps_bufs=6, repeat=1, msub=64, rg=None,
                 store_split=1, qsplit=False, rt_eng="sync"):
    """M=64 col-group variant: each 128-w block is two M=64 matmuls packed
    into psum partition halves via tile_position, so the rect narrows to
    127 cols (band at col (p mod 64) + d) and scratch traffic drops 33%."""
    import concourse.bass as bass
    import concourse.mybir as mybir
    import concourse.tile as tile
    from concourse import bacc

    nc = bacc.Bacc()
    left = nc.declare_dram_parameter("left", [rows * W, C], mybir.dt.float32,
                                     isOutput=False)
    right = nc.declare_dram_parameter("right", [rows * W, C], mybir.dt.float32,
                                      isOutput=False)
    out = nc.declare_dram_parameter("out", [rows * W, D], mybir.dt.bfloat16,
                                    isOutput=True)

    RG = rg or globals()["RG"]
    ng = rows // RG
    nblocks = RG * NBLK
    rect2 = msub + 63              # rect cols per block
    nsub = 128 // msub             # sub-matmuls per block
    bcols = nblocks * rect2
    SW = 95                        # stored cols per block under qsplit
    scols = nblocks * SW
    scr = [nc.dram_tensor(f"scr{g}", [128, scols if qsplit else bcols],
                          mybir.dt.bfloat16)
           for g in range(ng)]

    with tile.TileContext(nc) as tc:
        with (
            tc.tile_pool(name="lt", bufs=lt_bufs) as lt_pool,
            tc.tile_pool(name="rt", bufs=lt_bufs) as rt_pool,
            tc.tile_pool(name="rect", bufs=rect_bufs) as rect_pool,
            tc.tile_pool(name="ps", bufs=ps_bufs, space="PSUM") as psum_pool,
        ):
          for _rep in range(repeat):
            for g in range(ng):
                row0 = g * RG
                Lt = lt_pool.tile([CB, RG * W], mybir.dt.bfloat16, tag="lt")
                Rt = rt_pool.tile([CB, RG * W], mybir.dt.bfloat16, tag="rt")
                lsrc = left[row0 * W:(row0 + RG) * W, :].bitcast(mybir.dt.bfloat16)
                rsrc = right[row0 * W:(row0 + RG) * W, :].bitcast(mybir.dt.bfloat16)
                nc.sync.dma_start(Lt[:, :], lsrc, transpose=True)
                getattr(nc, rt_eng).dma_start(Rt[:, :], rsrc, transpose=True)

                Brect = rect_pool.tile([128, bcols], mybir.dt.bfloat16, tag="rect")
                for r in range(RG):
                    for half in range(NBLK // 2):
                        i0, i1 = 2 * half, 2 * half + 1
                        m0 = r * NBLK + i0
                        col0 = m0 * rect2
                        P = psum_pool.tile([128, 512], mybir.dt.float32,
                                           tag="ps")
                        for bi, i in enumerate((i0, i1)):
                            wl = i * 128            # row-local block base
                            pc = bi * rect2
                            rb = r * W
                            # sub-block s: w in [wl+s*msub, wl+(s+1)*msub),
                            # psum partitions [s*msub, (s+1)*msub), window
                            # w' in [wl+s*msub-64, wl+s*msub+rect2-65].
                            for s in range(nsub):
                                ws = wl + s * msub
                                p0, p1 = s * msub, (s + 1) * msub
                                if i == 0 and ws < 64:
                                    # w' < 0 head undefined: zero it, compute
                                    # the valid tail starting at w' = 0
                                    zc = 64 - ws
                                    nc.vector.memset(P[p0:p1, pc: pc + zc], 0.0)
                                    nc.tensor.matmul(
                                        P[p0:p1, pc + zc: pc + rect2],
                                        Lt[:, rb + ws: rb + ws + msub],
                                        Rt[:, rb: rb + rect2 - zc],
                                        start=True, stop=True,
                                        tile_position=(0, p0))
                                else:
                                    nc.tensor.matmul(
                                        P[p0:p1, pc: pc + rect2],
                                        Lt[:, rb + ws: rb + ws + msub],
                                        Rt[:, rb + ws - 64: rb + ws + rect2 - 64],
                                        start=True, stop=True,
                                        tile_position=(0, p0))
                        ev_dst = Brect[:, col0: col0 + 2 * rect2]
                        if half % 2 == 0:
                            nc.vector.tensor_copy(ev_dst, P[:, 0:2 * rect2])
                        else:
                            nc.scalar.copy(ev_dst, P[:, 0:2 * rect2])


                sap = scr[g][:, :]
                bap2 = Brect[:, :]
                oap = out[row0 * W:(row0 + RG) * W, :]
                if qsplit:
                    # store only the 95-col window each partition quarter's
                    # band needs: quarter t = partitions [32t, 32t+32),
                    # q = p mod 64 = 32*(t%2) + u, window cols
                    # [32*(t%2), 32*(t%2)+95) of each block ->
                    # scr[32t+u, SW*m + u + d] holds the band at col u+d.
                    for t in range(4):
                        ct = 32 * (t % 2)
                        bs = bass.AP(bap2.tensor,
                                     bap2.offset + 32 * t * bcols + ct,
                                     [[bcols, 32], [rect2, nblocks], [1, SW]])
                        sd = bass.AP(sap.tensor,
                                     sap.offset + 32 * t * scols,
                                     [[scols, 32], [SW, nblocks], [1, SW]])
                        getattr(nc, store_eng).dma_start(sd, bs)
                    for t in range(4):
                        srcp = bass.AP(sap.tensor,
                                       sap.offset + 32 * t * scols,
                                       [[scols + 1, 32], [SW, nblocks], [1, D]])
                        dstp = bass.AP(oap.tensor,
                                       oap.offset + 32 * t * D,
                                       [[D, 32], [128 * D, nblocks], [1, D]])
                        getattr(nc, diag_eng).dma_start(dstp, srcp)
                else:
                    # store rect (in store_split chunks of rows), then extract
                    # per (chunk, partition half): G[128m + msub*h + q, d'] =
                    # scr[msub*h + q, rect2*m + q + d']
                    rpc_ = RG // store_split          # rows per chunk
                    bpc = rpc_ * NBLK                 # blocks per chunk
                    ccols = bpc * rect2               # rect cols per chunk
                    for cch in range(store_split):
                        getattr(nc, store_eng).dma_start(
                            scr[g][:, cch * ccols:(cch + 1) * ccols],
                            Brect[:, cch * ccols:(cch + 1) * ccols])
                        for h in range(nsub):
                            src = bass.AP(sap.tensor,
                                          sap.offset + h * msub * bcols + cch * ccols,
                                          [[bcols + 1, msub], [rect2, bpc], [1, D]])
                            dst = bass.AP(oap.tensor,
                                          oap.offset + h * msub * D
                                          + cch * bpc * 128 * D,
                                          [[D, msub], [128 * D, bpc], [1, D]])
                            getattr(nc, diag_eng).dma_start(dst, src)

    nc.compile()
    return nc


def _prep(x, scale):
    # exact scale (power of two), then truncate to bf16-in-f32: the low 16
    # bits of every f32 become zero, so the bf16 *pair* view has +0.0 in the
    # garbage lanes.
    x = np.ascontiguousarray(x, dtype=np.float32) * scale
    return (x.view(np.uint32) & np.uint32(0xFFFF0000)).view(np.float32)


def make_in_maps(seed=0):
    """Random prepped per-core in_maps (bench harness helper)."""
    rng = np.random.default_rng(seed)
    lf = rng.standard_normal((ROWS, W, C), dtype=np.float32)
    rf = rng.standard_normal((ROWS, W, C), dtype=np.float32)
    lf = _prep(lf, 1.0 / C)
    rf = _prep(rf, 1.0)
    in_maps = []
    for k in range(N_CORES):
        sl = slice(k * ROWS_PER_CORE, (k + 1) * ROWS_PER_CORE)
        in_maps.append({
            "left": np.ascontiguousarray(lf[sl]).reshape(ROWS_PER_CORE * W, C),
            "right": np.ascontiguousarray(rf[sl]).reshape(ROWS_PER_CORE * W, C),
        })
    return in_maps


def in_map_to_rows(m):
    """Recover [ROWS_PER_CORE, W, C] f32 (prepped) views from an in_map."""
    lf = np.asarray(m["left"], dtype=np.float32).reshape(ROWS_PER_CORE, W, C)
    rf = np.asarray(m["right"], dtype=np.float32).reshape(ROWS_PER_CORE, W, C)
    return lf, rf


_NC_CACHE = {}


def kernel(left_feature, right_feature):
    from concourse.bass_utils import run_bass_kernel_spmd

    lf = _prep(left_feature, 1.0 / C).reshape(ROWS, W, C)
    rf = _prep(right_feature, 1.0).reshape(ROWS, W, C)

    if "nc" not in _NC_CACHE:
        _NC_CACHE["nc"] = build_nc_m64(store_eng="sync", diag_eng="sync")
    nc = _NC_CACHE["nc"]

    in_maps = []
    for k in range(N_CORES):
        sl = slice(k * ROWS_PER_CORE, (k + 1) * ROWS_PER_CORE)
        in_maps.append({
            "left": np.ascontiguousarray(lf[sl]).reshape(ROWS_PER_CORE * W, C),
            "right": np.ascontiguousarray(rf[sl]).reshape(ROWS_PER_CORE * W, C),
        })

    res = run_bass_kernel_spmd(nc, in_maps, core_ids=list(range(N_CORES)))

    out = np.empty((ROWS, W, D), dtype=np.float32)
    for k in range(N_CORES):
        g = res.results[k]["out"].astype(np.float32).reshape(ROWS_PER_CORE, W, D)
        out[k * ROWS_PER_CORE:(k + 1) * ROWS_PER_CORE] = g[:, :, ::-1]
    return out.reshape(B_FULL, H_FULL, W, D)



# revision 7
# speedup vs baseline: 986.4364x; 1.5352x over previous
"""Cost-volume kernel for Trainium2 (8 NeuronCores, data-parallel over B*H rows).

cost[b,h,w,d] = mean_c left[b,h,w,c] * right[b,h,w-(d+1),c], 0 where w-d-1 < 0
Shapes: B=4, H=256, W=512, C=64, D=64 (f32).

Strategy per core (128 independent (b,h) rows):
  - Host casts inputs to real bf16 and pre-transposes to [C, rows*W]
    (left pre-scaled by 1/C), halving input DMA traffic vs f32 and making
    loads plain contiguous DMAs (no transpose-DMA).  "right" is loaded into
    a per-row padded SBUF layout [C, rows*(64+W)] whose 64-col pads are
    zeroed on-chip, so every matmul window is uniform and w-d-1 < 0 yields
    exact zeros straight from the matmul.
  - TensorE: per 128-w block, two M=64/K=64/N=127 matmuls packed into psum
    partition halves via tile_position, rhs windows shifted by 64: psum
    [128, 127] rect per block with the band at col (p mod 64) + d'
    (d' = 63 - d).
  - DVE/ACT evict psum (f32 -> bf16) into a per-group SBUF rect; the rect
    is stored contiguously (full-rate DMA) to per-group DRAM scratch, and a
    DRAM->DRAM DMA with a sheared (flat-affine, +1-elem-per-row) source AP
    pulls out the band G[w, d'] = rect[p, 127*m + (p mod 64) + d'].
    (SBUF-side DMA descriptors cannot express the per-partition shear -
    DRAM-side flat addressing can.)
  - DMA issue is split across the two HWDGE queues (SP=sync, ACT=scalar).
  - Host flips d' -> d and casts bf16 -> f32 during unsharding.
"""

import numpy as np

N_CORES = 8
B_FULL, H_FULL, W, C = 4, 256, 512, 64
D = 64
ROWS = B_FULL * H_FULL           # 1024 independent rows
ROWS_PER_CORE = ROWS // N_CORES  # 128
NB = W // 128                    # 128-w blocks per row
RECT = 127                       # rect cols per block (band at q + d')
SEG = W + 64                     # per-row padded segment in Rt


def build_v3(rows=ROWS_PER_CORE, rg=8, lt_bufs=3, rect_bufs=3, ps_bufs=6,
             load_eng="sync", store_eng="sync", diag_eng="scalar",
             pad_eng="gpsimd", ev_engs=("vector", "scalar"), repeat=1,
             skip=()):
    import concourse.bass as bass
    import concourse.mybir as mybir
    import concourse.tile as tile
    from concourse import bacc

    nc = bacc.Bacc()
    left = nc.declare_dram_parameter("left", [C, rows * W], mybir.dt.bfloat16,
                                     isOutput=False)
    right = nc.declare_dram_parameter("right", [C, rows * W], mybir.dt.bfloat16,
                                      isOutput=False)
    out = nc.declare_dram_parameter("out", [rows * W, D], mybir.dt.bfloat16,
                                    isOutput=True)

    ng = rows // rg
    nblocks = rg * NB              # rect blocks per group
    bcols = nblocks * RECT         # rect buffer bf16 cols
    scr = [nc.dram_tensor(f"scr{g}", [128, bcols], mybir.dt.bfloat16)
           for g in range(ng)]

    with tile.TileContext(nc) as tc:
        with (
            tc.tile_pool(name="lt", bufs=lt_bufs) as lt_pool,
            tc.tile_pool(name="rt", bufs=lt_bufs) as rt_pool,
            tc.tile_pool(name="rect", bufs=rect_bufs) as rect_pool,
            tc.tile_pool(name="ps", bufs=ps_bufs, space="PSUM") as psum_pool,
        ):
          for _rep in range(repeat):
            for g in range(ng):
                c0 = g * rg * W
                Lt = lt_pool.tile([C, rg * W], mybir.dt.bfloat16, tag="lt")
                Rt = rt_pool.tile([C, rg * SEG], mybir.dt.bfloat16, tag="rt")
                if "loads" not in skip:
                    getattr(nc, load_eng).dma_start(Lt[:, :],
                                                    left[:, c0:c0 + rg * W])
                # zero the 64-col pads, then drop each row after its pad
                for r in range(rg):
                    getattr(nc, pad_eng).memset(
                        Rt[:, r * SEG: r * SEG + 64], 0.0)
                rap = Rt[:, :]
                rdst = bass.AP(rap.tensor, rap.offset + 64,
                               [[rg * SEG, C], [SEG, rg], [1, W]])
                rsap = right[:, c0:c0 + rg * W]
                rsrc = bass.AP(rsap.tensor, rsap.offset,
                               [[rows * W, C], [W, rg], [1, W]])
                getattr(nc, load_eng).dma_start(rdst, rsrc)

                Brect = rect_pool.tile([128, bcols], mybir.dt.bfloat16,
                                       tag="rect")
                for r in range(rg):
                    for half in range(NB // 2):
                        # two 128-w blocks share one psum tile (col halves);
                        # sub s covers w = 128i + 64s + q on partitions
                        # [64s, 64s+64); rhs window w' in [w0s - 64, w0s + 63)
                        P = psum_pool.tile([128, 2 * RECT], mybir.dt.float32,
                                           tag="ps")
                        for bi in range(2):
                            i = 2 * half + bi
                            for s in range(2):
                                nc.tensor.matmul(
                                    P[64 * s:64 * (s + 1),
                                      bi * RECT:(bi + 1) * RECT],
                                    Lt[:, r * W + 128 * i + 64 * s:
                                       r * W + 128 * i + 64 * (s + 1)],
                                    Rt[:, r * SEG + 128 * i + 64 * s:
                                       r * SEG + 128 * i + 64 * s + RECT],
                                    start=True, stop=True,
                                    tile_position=(0, 64 * s))
                        m0 = r * NB + 2 * half
                        ev = ev_engs[(r * (NB // 2) + half) % len(ev_engs)]
                        if ev == "vector":
                            nc.vector.tensor_copy(
                                Brect[:, m0 * RECT:(m0 + 2) * RECT], P[:, :])
                        else:
                            nc.scalar.copy(
                                Brect[:, m0 * RECT:(m0 + 2) * RECT], P[:, :])

                # rect -> DRAM scratch (contiguous, full-rate), then band
                # extraction via sheared DRAM->DRAM:
                # out[row0*W + 128m + 64h + q, d'] = scr[64h + q, 127m + q + d']
                sap = scr[g][:, :]
                oap = out[g * rg * W:(g + 1) * rg * W, :]
                if "store" not in skip:
                    getattr(nc, store_eng).dma_start(scr[g][:, :], Brect[:, :])
                for h in range(2 if "extract" not in skip else 0):
                    src = bass.AP(sap.tensor,
                                  sap.offset + 64 * h * bcols,
                                  [[bcols + 1, 64], [RECT, nblocks], [1, D]])
                    dst = bass.AP(oap.tensor,
                                  oap.offset + 64 * h * D,
                                  [[D, 64], [128 * D, nblocks], [1, D]])
                    getattr(nc, diag_eng).dma_start(dst, src)

    nc.compile()
    return nc


def _to_bf16_t(x, scale):
    """[rows, W, C] f32 -> [C, rows*W] bf16 (contiguous), optional scale."""
    import ml_dtypes
    if scale != 1.0:
        x = x * scale
    xt = np.ascontiguousarray(x.transpose(2, 0, 1)).astype(ml_dtypes.bfloat16)
    return xt.reshape(C, -1)


def make_in_maps(seed=0):
    """Random prepped per-core in_maps (bench harness helper)."""
    rng = np.random.default_rng(seed)
    lf = rng.standard_normal((ROWS, W, C), dtype=np.float32)
    rf = rng.standard_normal((ROWS, W, C), dtype=np.float32)
    in_maps = []
    for k in range(N_CORES):
        sl = slice(k * ROWS_PER_CORE, (k + 1) * ROWS_PER_CORE)
        in_maps.append({
            "left": _to_bf16_t(lf[sl], 1.0 / C),
            "right": _to_bf16_t(rf[sl], 1.0),
        })
    return in_maps


def in_map_to_rows(m):
    """Recover [ROWS_PER_CORE, W, C] f32 (prepped) arrays from an in_map."""
    lf = np.asarray(m["left"]).astype(np.float32)
    rf = np.asarray(m["right"]).astype(np.float32)
    lf = lf.reshape(C, ROWS_PER_CORE, W).transpose(1, 2, 0)
    rf = rf.reshape(C, ROWS_PER_CORE, W).transpose(1, 2, 0)
    return lf, rf


_NC_CACHE = {}


def kernel(left_feature, right_feature):
    from concourse.bass_utils import run_bass_kernel_spmd

    lf = np.asarray(left_feature, dtype=np.float32).reshape(ROWS, W, C)
    rf = np.asarray(right_feature, dtype=np.float32).reshape(ROWS, W, C)

    if "nc" not in _NC_CACHE:
        _NC_CACHE["nc"] = build_v3()
    nc = _NC_CACHE["nc"]

    in_maps = []
    for k in range(N_CORES):
        sl = slice(k * ROWS_PER_CORE, (k + 1) * ROWS_PER_CORE)
        in_maps.append({
            "left": _to_bf16_t(lf[sl], 1.0 / C),
            "right": _to_bf16_t(rf[sl], 1.0),
        })

    res = run_bass_kernel_spmd(nc, in_maps, core_ids=list(range(N_CORES)))

    out = np.empty((ROWS, W, D), dtype=np.float32)
    for k in range(N_CORES):
        g = res.results[k]["out"].astype(np.float32).reshape(
            ROWS_PER_CORE, W, D)
        out[k * ROWS_PER_CORE:(k + 1) * ROWS_PER_CORE] = g[:, :, ::-1]
    return out.reshape(B_FULL, H_FULL, W, D)
